# revision 10
# baseline (speedup 1.0000x reference)
"""Trainium2 Bass kernel for nn_GATModel (GATv2 on a bidirectional chain graph).

Key algebraic facts exploited (derived from the reference):
  * The reference's conv loop feeds x0 into EVERY layer, so only the LAST
    GATv2 layer (index L-1) affects the output.
  * x0 = x @ W_exp + b_exp + pe  never needs materializing:
        xl = x0 @ Wl + bl = x @ (W_exp@Wl) + [(b_exp+pe[n])@Wl + bl]
    i.e. a [64,256] matmul plus a per-node (n) bias.
  * The graph is a chain + self loops, so message passing is a 3-tap stencil
    (left / self / right) with a masked 3-way softmax per node.
  * a . leaky_relu(z) = 0.6*(a . z) + 0.4*(a . |z|)   (slope 0.2)
    and with ahat=|a| folded into the weight columns (positively homogeneous),
    a_h*|z_h| = sign(a_h)*|ztilde_h|.  So the nonlinear part is a signed sum
    of |ztilde| and the linear part is two per-node scalars (p, q).

Device computes (per 128-row tiles, col-major [h-part, row-free]):
  z_sigma (PSUM, via matmul accumulation incl. one-hot bias matmul)
  -> |z| (DVE tensor_scalar abs_max / ACT activation Abs, PSUM->SBUF bf16)
  -> t_sigma = sum_h sign(a_h)*|ztilde| (PE matmul with sign vector)
  plus p = x@ (Wl'a), q = x @ (Wr'a), y = x @ (Wl' W_fc)  (one small matmul).
Host finishes: logits, masks, 3-way softmax, alpha-weighted message pooling,
final fc.  (All heavy O(B*N*H) math is on device.)
"""

import os
import sys

sys.path.insert(0, "/opt/trn_rl_repo")

from contextlib import ExitStack  # noqa: E402

import ml_dtypes  # noqa: E402
import numpy as np  # noqa: E402

import concourse.bass as bass  # noqa: E402
import concourse.tile as tile  # noqa: E402
from concourse import bacc, mybir  # noqa: E402
from concourse.bass_utils import run_bass_kernel_spmd  # noqa: E402

BF16 = mybir.dt.bfloat16
F32 = mybir.dt.float32
NPBF16 = ml_dtypes.bfloat16

B, N, IN, H, L, C = 2048, 100, 64, 256, 3, 3
NEG = 0.2
NCORES = 8
BC = B // NCORES            # 256 graphs per core
ROWS = BC * N               # 25600 rows per core
CH_ELEMS = 5
CHF = CH_ELEMS * N          # 500 rows per chunk
NFULL = BC // CH_ELEMS      # 51 full chunks
REM_ELEMS = BC - NFULL * CH_ELEMS   # 1 leftover graph
CHUNKS = [(i * CHF, CHF) for i in range(NFULL)]
if REM_ELEMS:
    CHUNKS.append((NFULL * CHF, REM_ELEMS * N))

LAST_RESULTS = None  # set by kernel() for test harness inspection


def _make_pe_np(n, d):
    pos = np.arange(n, dtype=np.float32)[:, None]
    div = np.exp(
        np.arange(0, d, 2, dtype=np.float32) * (-np.log(np.float32(10000.0)) / d)
    )
    pe = np.zeros((n, d), dtype=np.float32)
    pe[:, 0::2] = np.sin(pos * div)
    pe[:, 1::2] = np.cos(pos * div)
    return pe


def _route_is_dve(sigma, chunk_idx):
    # Balance the PSUM->SBUF abs pass between VectorE (4/9) and ScalarE (5/9).
    return ((sigma + 3 * chunk_idx) % 9) < 4


_PROG_CACHE = None


def _build_program():
    """Build the (shape-only) Bass program once; weights arrive via in_maps."""
    nc = bacc.Bacc(
        "TRN2",
        target_bir_lowering=False,
        debug=False,
        enable_asserts=False,
        num_devices=NCORES,
    )

    d_in = {}

    def din(name, shape, dt):
        d_in[name] = nc.dram_tensor(name, list(shape), dt, kind="ExternalInput").ap()
        return d_in[name]

    xT = din("xT", (64, ROWS), BF16)
    S_lr0 = din("S_lr0", (128, 128), BF16)
    S_lr1 = din("S_lr1", (128, 128), BF16)
    S_rl0 = din("S_rl0", (128, 128), BF16)
    S_rl1 = din("S_rl1", (128, 128), BF16)
    S_self = din("S_self", (128, 128), BF16)
    Wpqy = din("Wpqy", (128, 8), BF16)
    COEF = din("COEF", (128, 2), F32)
    SEL = din("SEL", (100, CHF), BF16)
    D_tiles_dram = {
        (s, b): din(f"D_{s}{b}", (100, 128), BF16)
        for s in ("l", "r", "s")
        for b in (0, 1)
    }
    outs_dram = nc.dram_tensor("outs", [8, ROWS], F32, kind="ExternalOutput").ap()

    with tile.TileContext(nc) as tc, ExitStack() as ctx:
        cpool = ctx.enter_context(tc.tile_pool(name="consts", bufs=1))
        x3pool = ctx.enter_context(tc.tile_pool(name="x3", bufs=1))
        zpool = ctx.enter_context(
            tc.tile_pool(name="z", bufs=1, space=bass.MemorySpace.PSUM)
        )
        tbpool = ctx.enter_context(
            tc.tile_pool(name="tb", bufs=2, space=bass.MemorySpace.PSUM)
        )
        wpool = ctx.enter_context(tc.tile_pool(name="w", bufs=2))
        spool = ctx.enter_context(tc.tile_pool(name="stage", bufs=2))

        def cload(name, dram_ap, shape, dt):
            t = cpool.tile(list(shape), dt, tag=f"c_{name}")
            nc.sync.dma_start(t[:], dram_ap[:])
            return t

        S_lr = [cload("slr0", S_lr0, (128, 128), BF16),
                cload("slr1", S_lr1, (128, 128), BF16)]
        S_rl = [cload("srl0", S_rl0, (128, 128), BF16),
                cload("srl1", S_rl1, (128, 128), BF16)]
        S_sf = cload("ssf", S_self, (128, 128), BF16)
        Wpq = cload("wpqy", Wpqy, (128, 8), BF16)
        CO = cload("coef", COEF, (128, 2), F32)
        SE = cload("sel", SEL, (100, CHF), BF16)
        D = {k: cload(f"d{k[0]}{k[1]}", v, (100, 128), BF16)
             for k, v in D_tiles_dram.items()}

        # x3: [0:64, c] = xT[:, c-1] (shifted), [64:128, c] = xT[:, c]
        x3 = x3pool.tile([128, ROWS + 2], BF16)
        nc.vector.memset(x3[:, 0:1], 0.0)
        nc.vector.memset(x3[:, ROWS : ROWS + 2], 0.0)
        nc.sync.dma_start(x3[64:128, 0:ROWS], xT[:, :])
        nc.sync.dma_start(x3[0:64, 1 : ROWS + 1], xT[:, :])

        for ci, (c0, F) in enumerate(CHUNKS):
            zt = {}
            # ---- z production: bias matmul (start) + data matmul (stop) ----
            for b in (0, 1):
                zl = zpool.tile([128, F], F32, tag=f"zl{b}")
                nc.tensor.matmul(zl[:], D[("l", b)][:], SE[:, 0:F],
                                 start=True, stop=False)
                nc.tensor.matmul(zl[:], S_lr[b][:], x3[:, c0 : c0 + F],
                                 start=False, stop=True)
                zt[("l", b)] = zl
            for b in (0, 1):
                zr = zpool.tile([128, F], F32, tag=f"zr{b}")
                nc.tensor.matmul(zr[:], D[("r", b)][:], SE[:, 0:F],
                                 start=True, stop=False)
                nc.tensor.matmul(zr[:], S_rl[b][:], x3[:, c0 + 1 : c0 + F + 1],
                                 start=False, stop=True)
                zt[("r", b)] = zr
            zs0 = zpool.tile([128, F], F32, tag="zs0")
            nc.tensor.matmul(zs0[:], D[("s", 0)][:], SE[:, 0:F],
                             start=True, stop=False)
            nc.tensor.matmul(zs0[:], S_sf[0:64, :], x3[0:64, c0 + 1 : c0 + F + 1],
                             start=False, stop=True)
            zt[("s", 0)] = zs0
            zs1 = zpool.tile([128, F], F32, tag="zs1")
            nc.tensor.matmul(zs1[:], D[("s", 1)][:], SE[:, 0:F],
                             start=True, stop=False)
            nc.tensor.matmul(zs1[:], S_sf[64:128, :], x3[64:128, c0 : c0 + F],
                             start=False, stop=True)
            zt[("s", 1)] = zs1

            # ---- |z| crossing PSUM -> SBUF (bf16), split DVE / ACT ----
            wt = {}
            for si, s in enumerate(("l", "r", "s")):
                for b in (0, 1):
                    w = wpool.tile([128, F], F32, tag=f"w{s}{b}")
                    if _route_is_dve(si, ci):
                        # |z| = clear fp32 sign bit (exact, 1x from PSUM)
                        nc.vector.tensor_scalar(
                            w[:].bitcast(mybir.dt.int32),
                            zt[(s, b)][:].bitcast(mybir.dt.int32),
                            0x7FFFFFFF,
                            None,
                            mybir.AluOpType.bitwise_and,
                        )
                    else:
                        nc.scalar.activation(
                            w[:], zt[(s, b)][:], mybir.ActivationFunctionType.Abs
                        )
                    wt[(s, b)] = w

            # ---- t_sigma = sum_h sign(a_h) * |ztilde|  (M=1 matmuls) ----
            # t_sigma rows at 32-aligned PSUM partitions (tile_position rule)
            tb = tbpool.tile([128, F], F32, tag="tb")
            for si, s in enumerate(("l", "r", "s")):
                p0 = 32 * si
                nc.tensor.matmul(tb[p0 : p0 + 1, 0:F], CO[:, 0:1], wt[(s, 0)][:],
                                 start=True, stop=False)
                nc.tensor.matmul(tb[p0 : p0 + 1, 0:F], CO[:, 1:2], wt[(s, 1)][:],
                                 start=False, stop=True)
            # ---- p, q, y rows at partitions 96..100 ----
            nc.tensor.matmul(tb[96:101, 0:F], Wpq[64:128, 0:5],
                             x3[64:128, c0 : c0 + F], start=True, stop=True,
                             tile_position=(64, 96))

            # ---- evacuate whole bank + strided DMA out ----
            st = spool.tile([128, F], F32, tag="st")
            if ci % 2 == 0:
                nc.vector.tensor_copy(st[:], tb[:, 0:F])
            else:
                nc.scalar.copy(st[:], tb[:, 0:F])
            nc.sync.dma_start(outs_dram[0:3, c0 : c0 + F], st[0:96:32, 0:F])
            nc.sync.dma_start(outs_dram[3:8, c0 : c0 + F], st[96:101, 0:F])

    nc.compile()
    return nc


def _get_program():
    global _PROG_CACHE
    if _PROG_CACHE is None:
        _PROG_CACHE = _build_program()
    return _PROG_CACHE


def kernel(x, W_exp, b_exp, W_l, b_l, W_r, b_r, att, bias, W_fc, b_fc):
    global LAST_RESULTS
    x = np.asarray(x, dtype=np.float32)
    W_exp = np.asarray(W_exp, np.float32)
    b_exp = np.asarray(b_exp, np.float32)
    W_l = np.asarray(W_l, np.float32)
    b_l = np.asarray(b_l, np.float32)
    W_r = np.asarray(W_r, np.float32)
    b_r = np.asarray(b_r, np.float32)
    att = np.asarray(att, np.float32)
    bias = np.asarray(bias, np.float32)
    W_fc = np.asarray(W_fc, np.float32)
    b_fc = np.asarray(b_fc, np.float32)

    lw = L - 1  # only the last conv layer matters
    pe = _make_pe_np(N, H)
    a = att[lw]
    s = np.where(a >= 0.0, 1.0, -1.0).astype(np.float32)
    ahat = np.abs(a)

    Wl_full = W_exp @ W_l[lw]                     # [64,256]
    Wr_full = W_exp @ W_r[lw]
    cl = (b_exp + pe) @ W_l[lw] + b_l[lw]         # [100,256]
    cr = (b_exp + pe) @ W_r[lw] + b_r[lw]

    Wtl = Wl_full * ahat[None, :]                 # ahat-folded
    Wtr = Wr_full * ahat[None, :]
    ctl = cl * ahat[None, :]
    ctr = cr * ahat[None, :]

    # stationaries [K,M]: K = concat feature dim, M = h-block columns
    def blk(Wm, b):
        return Wm[:, b * 128 : (b + 1) * 128]

    def bf(arr):
        return np.ascontiguousarray(arr.astype(NPBF16))

    consts = {}
    for b in (0, 1):
        consts[f"S_lr{b}"] = bf(np.concatenate([blk(Wtl, b), blk(Wtr, b)], axis=0))
        consts[f"S_rl{b}"] = bf(np.concatenate([blk(Wtr, b), blk(Wtl, b)], axis=0))
    Wts = Wtl + Wtr
    consts["S_self"] = bf(np.concatenate([blk(Wts, 0), blk(Wts, 1)], axis=0))

    # D bias tiles [100,128]: per-dst-node bias of ztilde for each edge type
    ctl_m1 = np.vstack([np.zeros((1, H), np.float32), ctl[:-1]])   # ctl[n-1]
    ctl_p1 = np.vstack([ctl[1:], np.zeros((1, H), np.float32)])    # ctl[n+1]
    Dfull = {
        "l": ctl_m1 + ctr,
        "r": ctl_p1 + ctr,
        "s": ctl + ctr,
    }
    for sname, Dm in Dfull.items():
        for b in (0, 1):
            consts[f"D_{sname}{b}"] = bf(Dm[:, b * 128 : (b + 1) * 128])

    # p/q/y weights: [64, 5] at partitions 64:128 of a [128,8] tile
    wp = Wl_full @ a                                # [64]
    wq = Wr_full @ a
    Wy = Wl_full @ W_fc                             # [64,3]
    Wpqy = np.zeros((128, 8), np.float32)
    Wpqy[64:, 0] = wp
    Wpqy[64:, 1] = wq
    Wpqy[64:, 2:5] = Wy
    consts["Wpqy"] = bf(Wpqy)

    COEF = np.zeros((128, 2), np.float32)
    COEF[:, 0] = s[0:128]
    COEF[:, 1] = s[128:256]
    consts["COEF"] = np.ascontiguousarray(COEF)

    SEL = np.zeros((100, CHF), np.float32)
    for j in range(CHF):
        SEL[j % 100, j] = 1.0
    consts["SEL"] = bf(SEL)

    # per-core inputs
    xr = x.reshape(NCORES, ROWS, IN)
    in_maps = []
    for c in range(NCORES):
        m = dict(consts)
        m["xT"] = bf(xr[c].T)                      # [64, ROWS]
        in_maps.append(m)

    nc = _get_program()
    res = run_bass_kernel_spmd(
        nc,
        in_maps,
        core_ids=list(range(NCORES)),
    )
    LAST_RESULTS = res

    # ---------------- host tail ----------------
    cp = cl @ a                                               # [100]
    cq = cr @ a
    cy = cl @ W_fc                                            # [100,3]
    n_of_r = np.tile(np.arange(N), BC)                        # [ROWS]

    out_all = np.empty((B, C), np.float32)
    for c in range(NCORES):
        o = np.asarray(res.results[c]["outs"], np.float32)    # [8, ROWS]
        t_l, t_r, t_s = o[0], o[1], o[2]
        P, Q = o[3], o[4]
        Yd = o[5:8].T                                         # [ROWS,3]

        Pb = P + cp[n_of_r]                                   # a.xl per row
        Qb = Q + cq[n_of_r]                                   # a.xr per row
        Y = Yd + cy[n_of_r]                                   # xl @ W_fc per row

        Pb_m1 = np.roll(Pb, 1)                                # P at source row r-1
        Pb_p1 = np.roll(Pb, -1)

        lg_l = 0.6 * (Pb_m1 + Qb) + 0.4 * t_l
        lg_r = 0.6 * (Pb_p1 + Qb) + 0.4 * t_r
        lg_s = 0.6 * (Pb + Qb) + 0.4 * t_s

        lg_l = np.where(n_of_r == 0, -np.inf, lg_l)
        lg_r = np.where(n_of_r == N - 1, -np.inf, lg_r)

        mx = np.maximum(np.maximum(lg_l, lg_r), lg_s)
        el = np.exp(lg_l - mx)
        er = np.exp(lg_r - mx)
        es = np.exp(lg_s - mx)
        den = el + er + es
        al, ar, asf = el / den, er / den, es / den

        Y_m1 = np.roll(Y, 1, axis=0)
        Y_p1 = np.roll(Y, -1, axis=0)
        msgs = al[:, None] * Y_m1 + ar[:, None] * Y_p1 + asf[:, None] * Y
        pooled = msgs.reshape(BC, N, C).sum(axis=1)
        out_all[c * BC : (c + 1) * BC] = (
            pooled + N * (bias[lw] @ W_fc)[None, :] + b_fc[None, :]
        )
    return out_all
